# revision 23
# baseline (speedup 1.0000x reference)
"""Multi-head attention block on 8 Trainium2 NeuronCores.

Problem: B=4, N=2048, C=768, H=12, HD=64 (f32).
  qkv = x @ w_qkv + b_qkv ; attn = softmax(q*k^T/8) ; out = (attn@v) @ w_proj + b_proj

Sharding: data-parallel over batch (4) x tensor-parallel over heads (2 groups
of 6 heads). Core c handles batch c//2, head-group c%2. Each core computes a
partial projection output [N, C]; the host sums the two head-group partials
per batch and adds b_proj.

v3 design. The run is scalar-engine(exp)-bound; everything is organized to
keep ACT saturated with F=1024 exps while the PE keeps pace:
  - host pre-casts x / weights to bf16 (halves input DMA, kills device casts)
  - x^T produced by 6 xbar DMA-transposes straight from DRAM (no PE
    transposes, no staging copies)
  - q^T/k^T in [pair*128, N] layout (head pair p at partitions 2p*64..);
    the two heads of a pair are emitted adjacently so their K=64 row-tiled
    scores matmuls run concurrently on the PE (partitions 0-63 / 64-127)
  - exp over [128, 1024] PSUM (2 banks) in one ACT instruction
  - attn@V accumulates [65, 1024] PSUM per head (ones column in V gives the
    softmax denominator in row 64); PSUM budget: 4 banks scores ping-pong +
    4 banks av = 8
  - at pair end av is evacuated to SBUF f32 on scalar+vector in parallel
    (frees the av banks fast); normalization trails one pair behind:
    denominator reciprocal (reciprocal_approx_fast at partition 0), K=1
    ones-matmul broadcast, DVE multiply into pair-stacked o2 [128, N] bf16
  - out-projection contracts K=128 per head pair (o2 stacked), [128, 768]
    PSUM, 4 output tiles batched per DMA
"""

import numpy as np

from concourse import bacc, bass, bass_utils, tile
from concourse import mybir

B, N, C, H, HD = 4, 2048, 768, 12, 64
SCALE = HD ** -0.5
P = 128
NT = N // P           # 16 key/n tiles
CT = C // P           # 6 contraction tiles over C
HPC = 6               # heads per core
NPAIR = 3             # head pairs per core
QC = 1024             # attention q-chunk
NCH = N // QC         # 2 q-chunks
VW = 65               # V columns per head incl. ones column
VWP = 80              # padded per-head V block stride
F32 = mybir.dt.float32
F32R = mybir.dt.float32r
BF16 = mybir.dt.bfloat16
EXP = mybir.ActivationFunctionType.Exp

_CACHE = {}


def build_program(mm_dt=BF16):
    MMDT = mm_dt
    nc = bacc.Bacc("TRN2", target_bir_lowering=False, debug=False, num_devices=8)

    x_d = nc.dram_tensor("x", [N, C], MMDT, kind="ExternalInput")
    wqk_d = nc.dram_tensor("wqk", [C, 2 * NPAIR * P], MMDT, kind="ExternalInput")
    wv_d = nc.dram_tensor("wv", [C, HPC * HD], MMDT, kind="ExternalInput")
    wp_d = nc.dram_tensor("wp", [NPAIR * P, C], MMDT, kind="ExternalInput")
    bqk_d = nc.dram_tensor("bqk", [P, CT], F32, kind="ExternalInput")
    bv_d = nc.dram_tensor("bv", [1, HPC * HD], MMDT, kind="ExternalInput")
    out_d = nc.dram_tensor("out", [N, C], F32, kind="ExternalOutput")

    with tile.TileContext(nc) as tc, nc.allow_low_precision(
            reason="bf16 matmul pipeline, approx reciprocal for softmax denom"):
        with (
            tc.tile_pool(name="const", bufs=1) as cpool,
            tc.tile_pool(name="persist", bufs=1) as pp,
        ):
            # x^T first: everything downstream depends on it. Each block is
            # transposed in n-halves so the first v/k matmuls (which touch
            # only early n-tiles) can start after half the transfer.
            xT = [pp.tile([P, N], MMDT, name=f"xT{ct}", tag=f"xT{ct}")
                  for ct in range(CT)]
            for ct in range(CT):
                nc.sync.dma_start(xT[ct][:, 0:N // 2],
                                  x_d[0:N // 2, ct * P:(ct + 1) * P],
                                  transpose=True)
            wv_sb = []
            for ct in range(CT):
                tv = pp.tile([P, HPC * HD], MMDT, name=f"wv{ct}", tag=f"wv{ct}")
                nc.sync.dma_start(tv[:], wv_d[ct * P:(ct + 1) * P, :])
                wv_sb.append(tv)
            bv = cpool.tile([1, HPC * HD], MMDT, name="bv", tag="bv")
            nc.sync.dma_start(bv[:], bv_d[:])
            for ct in range(CT):
                nc.sync.dma_start(xT[ct][:, N // 2:N],
                                  x_d[N // 2:N, ct * P:(ct + 1) * P],
                                  transpose=True)
            w_sb = []
            for ct in range(CT):
                t = pp.tile([P, 2 * NPAIR * P], MMDT, name=f"w{ct}", tag=f"w{ct}")
                nc.sync.dma_start(t[:], wqk_d[ct * P:(ct + 1) * P, :])
                w_sb.append(t)
            bqk = cpool.tile([P, CT], F32, name="bqk", tag="bqk")
            nc.sync.dma_start(bqk[:], bqk_d[:])
            wp_sb = []
            for p in range(NPAIR):
                t = pp.tile([P, C], MMDT, name=f"wp{p}", tag=f"wp{p}")
                nc.sync.dma_start(t[:], wp_d[p * P:(p + 1) * P, :])
                wp_sb.append(t)

            onesb = cpool.tile([1, 512], MMDT, name="onesb", tag="onesb")
            nc.gpsimd.memset(onesb[:], 1.0)

            qT = [pp.tile([P, N], MMDT, name=f"q{i}", tag=f"q{i}")
                  for i in range(NPAIR)]
            kT = [pp.tile([P, N], MMDT, name=f"k{i}", tag=f"k{i}")
                  for i in range(NPAIR)]
            v_sb = pp.tile([P, NT * HPC * VWP], MMDT, name="v", tag="v")
            # ones columns: memset everything to 1, V writes leave col 64 = 1
            nc.gpsimd.memset(v_sb[:], 1.0)
            o2_sb = [pp.tile([P, N], MMDT, name=f"o2{p}", tag=f"o2{p}")
                     for p in range(NPAIR)]

            # ---------------- phase A: q/k/v projections ----------------
            with (
                tc.tile_pool(name="vps", bufs=2, space="PSUM") as v_ps,
                tc.tile_pool(name="qkps", bufs=2, space="PSUM") as qk_ps,
            ):
                for j in range(4):
                    for ntl in range(4):
                        nt = 4 * j + ntl
                        # V rows for this n-tile (+ bias via K=1 matmul)
                        vps = v_ps.tile([P, HPC * HD], F32, name="vps", tag="vps")
                        for ct in range(CT):
                            nc.tensor.matmul(
                                vps[:],
                                xT[ct][:, nt * P:(nt + 1) * P],
                                wv_sb[ct][:],
                                start=(ct == 0), stop=False)
                        nc.tensor.matmul(
                            vps[:], onesb[0:1, 0:P], bv[:], start=False, stop=True)
                        nc.vector.tensor_copy(
                            v_sb[:].rearrange("p (t w) -> p t w", w=VWP)
                                [:, nt * HPC:(nt + 1) * HPC, 0:HD],
                            vps[:].rearrange("p (h w) -> p h w", w=HD))
                    # q^T / k^T for this n-chunk of 512
                    for colt in range(2 * NPAIR):
                        qps = qk_ps.tile([P, 512], F32, name="qkp", tag="qkp")
                        for ct in range(CT):
                            nc.tensor.matmul(
                                qps[:],
                                w_sb[ct][:, colt * P:(colt + 1) * P],
                                xT[ct][:, j * 512:(j + 1) * 512],
                                start=(ct == 0), stop=(ct == CT - 1))
                        dest = qT[colt] if colt < NPAIR else kT[colt - NPAIR]
                        nc.vector.tensor_scalar_add(
                            dest[:, j * 512:(j + 1) * 512], qps[:],
                            bqk[:, colt:colt + 1])

            # ---------------- phase B: attention ----------------
            with (
                tc.tile_pool(name="sps", bufs=2, space="PSUM") as s_ps,
                tc.tile_pool(name="avps", bufs=2, space="PSUM") as av_ps,
                tc.tile_pool(name="exsb", bufs=3) as ex_pool,
                tc.tile_pool(name="o2u", bufs=4) as o2u_pool,
                tc.tile_pool(name="rec", bufs=4) as rec_pool,
            ):
                def heartbeat(av0, n):
                    """Dummy K=1 matmuls into the unused partition-96 row of
                    an av tile (start=False never clears has_written; the row
                    is never read). Pure PE activity that keeps the HAM clock
                    gate from demoting across pair-boundary duty dips."""
                    for _ in range(n):
                        nc.tensor.matmul(
                            av0[96:97, 0:512], onesb[0:1, 0:1],
                            onesb[0:1, :], start=False, stop=True,
                            skip_group_check=True, tile_position=(0, 96))

                def attn_body(c, p):
                    """kt loop for q-chunk c, head pair p. Returns state for
                    the trailing normalization."""
                    av = [av_ps.tile([P, QC], F32, name="av", tag="av")
                          for _ in range(2)]
                    heartbeat(av[0], 6)
                    for kt in range(NT):
                        sps = [s_ps.tile([P, QC], F32, name="s", tag="s")
                               for _ in range(2)]
                        # both heads' scores adjacent: row groups 0-63/64-127
                        # run concurrently on the PE
                        for half in range(2):
                            for par in range(2):
                                rows = slice(par * HD, (par + 1) * HD)
                                fsl = slice(half * 512, (half + 1) * 512)
                                nc.tensor.matmul(
                                    sps[par][:, fsl],
                                    kT[p][rows, kt * P:(kt + 1) * P],
                                    qT[p][rows, c * QC + half * 512:
                                          c * QC + (half + 1) * 512],
                                    start=True, stop=True)
                        exs = []
                        for par in range(2):
                            ex = ex_pool.tile([P, QC], MMDT, name="ex", tag="ex")
                            nc.scalar.activation(ex[:], sps[par][:], EXP)
                            exs.append(ex)
                        if kt < 3:
                            heartbeat(av[0], 5)
                        for par in range(2):
                            h = 2 * p + par
                            vcol = (kt * HPC + h) * VWP
                            for half in range(2):
                                fsl = slice(half * 512, (half + 1) * 512)
                                nc.tensor.matmul(
                                    av[par][0:VW, fsl],
                                    v_sb[:, vcol:vcol + VW],
                                    exs[par][:, fsl],
                                    start=(kt == 0), stop=(kt == NT - 1))
                    # evacuate av fast on two engines; denominator row straight
                    # from PSUM so the normalize chain starts immediately
                    o2u, dens = [], []
                    for par in range(2):
                        eng = nc.scalar if par == 0 else nc.vector
                        den = rec_pool.tile([1, QC], F32, name="den", tag="den")
                        t = o2u_pool.tile([HD, QC], F32, name="o2u", tag="o2u")
                        if par == 0:
                            eng.copy(den[:], av[par][HD:VW, :])
                            eng.copy(t[:], av[par][0:HD, :])
                        else:
                            eng.tensor_copy(den[:], av[par][HD:VW, :])
                            eng.tensor_copy(t[:], av[par][0:HD, :])
                        dens.append(den)
                        o2u.append(t)
                    return o2u, dens

                def normalize(c, p, o2u, dens):
                    """Softmax-normalize o2u into o2_sb (pair-stacked bf16).
                    The reciprocal row is replicated across 64 partitions on
                    the (idle) gpsimd engine — no PE/PSUM involvement, so
                    nothing here can gate the attention pipeline."""
                    qsl = slice(c * QC, (c + 1) * QC)
                    for par in range(2):
                        rec = rec_pool.tile([1, QC], F32, name="rec", tag="rec")
                        nc.vector.reciprocal_approx_fast(rec[:], dens[par][:])
                        rbc = rec_pool.tile([HD, QC], F32, name="rbc",
                                            tag="rbc")
                        nc.gpsimd.partition_broadcast(rbc[:], rec[0:1, :])
                        nc.vector.tensor_tensor(
                            o2_sb[p][par * HD:(par + 1) * HD, qsl],
                            o2u[par][:], rbc[:],
                            op=mybir.AluOpType.mult)

                pending = None
                for c in range(NCH):
                    for p in range(NPAIR):
                        state = attn_body(c, p)
                        if pending is not None:
                            normalize(*pending)
                        pending = (c, p, *state)
                assert pending is not None
                normalize(*pending)

            # ---------------- phase C: out-projection ----------------
            with (
                tc.tile_pool(name="pps", bufs=2, space="PSUM") as p_ps,
                tc.tile_pool(name="outsb", bufs=2) as out_pool,
            ):
                for j in range(4):
                    out4 = out_pool.tile([P, 4 * C], F32, name="out4",
                                         tag="out4")
                    for tl in range(4):
                        t = 4 * j + tl
                        pps = p_ps.tile([P, C], F32, name="pps", tag="pps")
                        for n0, nw in ((0, 512), (512, C - 512)):
                            for p in range(NPAIR):
                                nc.tensor.matmul(
                                    pps[:, n0:n0 + nw],
                                    o2_sb[p][:, t * P:(t + 1) * P],
                                    wp_sb[p][:, n0:n0 + nw],
                                    start=(p == 0), stop=(p == NPAIR - 1))
                        nc.vector.tensor_copy(
                            out4[:, tl * C:(tl + 1) * C], pps[:])
                    nc.sync.dma_start(
                        out_d[j * 512:(j + 1) * 512, :]
                        .rearrange("(a p) c -> p a c", p=P),
                        out4[:].rearrange("p (a c) -> p a c", c=C))

    nc.compile()
    return nc


def _get_program(mm_dt=BF16):
    key = str(mm_dt)
    if key not in _CACHE:
        _CACHE[key] = build_program(mm_dt)
    return _CACHE[key]


def make_in_maps(x, w_qkv, b_qkv, w_proj):
    import ml_dtypes
    bf = ml_dtypes.bfloat16
    x = np.asarray(x, np.float32)
    w_qkv = np.asarray(w_qkv, np.float32)
    b_qkv = np.asarray(b_qkv, np.float32)
    w_proj = np.asarray(w_proj, np.float32)
    in_maps = []
    for c in range(8):
        b, hg = divmod(c, 2)
        hsl = slice(hg * HPC * HD, (hg + 1) * HPC * HD)
        wq = w_qkv[:, 0:C][:, hsl] * SCALE
        wk = w_qkv[:, C:2 * C][:, hsl]
        wv = w_qkv[:, 2 * C:3 * C][:, hsl]
        wqk_in = np.ascontiguousarray(
            np.concatenate([wq, wk], axis=1).astype(bf))
        bq = b_qkv[0:C][hsl] * SCALE
        bk = b_qkv[C:2 * C][hsl]
        bvv = b_qkv[2 * C:3 * C][hsl]
        bqk_in = np.ascontiguousarray(
            np.concatenate([bq, bk]).reshape(CT, P).T)
        wp_in = np.ascontiguousarray(w_proj[hsl, :].astype(bf))
        in_maps.append({
            "x": np.ascontiguousarray(x[b].astype(bf)),
            "wqk": wqk_in,
            "wv": np.ascontiguousarray(wv.astype(bf)),
            "wp": wp_in,
            "bqk": bqk_in,
            "bv": np.ascontiguousarray(bvv.reshape(1, HPC * HD).astype(bf)),
        })
    return in_maps


def run(x, w_qkv, b_qkv, w_proj, b_proj, mm_dt=BF16, **run_kwargs):
    nc = _get_program(mm_dt)
    in_maps = make_in_maps(x, w_qkv, b_qkv, w_proj)
    res = bass_utils.run_bass_kernel_spmd(
        nc, in_maps, core_ids=list(range(8)), **run_kwargs)
    y = np.empty((B, N, C), np.float32)
    for b in range(B):
        y[b] = res.results[2 * b]["out"] + res.results[2 * b + 1]["out"]
    y += np.asarray(b_proj, np.float32)
    return y, res


def kernel(x, w_qkv, b_qkv, w_proj, b_proj):
    y, _ = run(x, w_qkv, b_qkv, w_proj, b_proj)
    return y


# revision 24
# speedup vs baseline: 1.0346x; 1.0346x over previous
"""Multi-head attention block on 8 Trainium2 NeuronCores.

Problem: B=4, N=2048, C=768, H=12, HD=64 (f32).
  qkv = x @ w_qkv + b_qkv ; attn = softmax(q*k^T/8) ; out = (attn@v) @ w_proj + b_proj

Sharding: data-parallel over batch (4) x tensor-parallel over heads (2 groups
of 6 heads). Core c handles batch c//2, head-group c%2. Each core computes a
partial projection output [N, C]; the host sums the two head-group partials
per batch and adds b_proj.

v3 design. The run is scalar-engine(exp)-bound; everything is organized to
keep ACT saturated with F=1024 exps while the PE keeps pace:
  - host pre-casts x / weights to bf16 (halves input DMA, kills device casts)
  - x^T produced by 6 xbar DMA-transposes straight from DRAM (no PE
    transposes, no staging copies)
  - q^T/k^T in [pair*128, N] layout (head pair p at partitions 2p*64..);
    the two heads of a pair are emitted adjacently so their K=64 row-tiled
    scores matmuls run concurrently on the PE (partitions 0-63 / 64-127)
  - exp over [128, 1024] PSUM (2 banks) in one ACT instruction
  - attn@V accumulates [65, 1024] PSUM per head (ones column in V gives the
    softmax denominator in row 64); PSUM budget: 4 banks scores ping-pong +
    4 banks av = 8
  - at pair end av is evacuated to SBUF f32 on scalar+vector in parallel
    (frees the av banks fast); normalization trails one pair behind:
    denominator reciprocal (reciprocal_approx_fast at partition 0), K=1
    ones-matmul broadcast, DVE multiply into pair-stacked o2 [128, N] bf16
  - out-projection contracts K=128 per head pair (o2 stacked), [128, 768]
    PSUM, 4 output tiles batched per DMA
"""

import numpy as np

from concourse import bacc, bass, bass_utils, tile
from concourse import mybir

B, N, C, H, HD = 4, 2048, 768, 12, 64
SCALE = HD ** -0.5
P = 128
NT = N // P           # 16 key/n tiles
CT = C // P           # 6 contraction tiles over C
HPC = 6               # heads per core
NPAIR = 3             # head pairs per core
QC = 1024             # attention q-chunk
NCH = N // QC         # 2 q-chunks
VW = 65               # V columns per head incl. ones column
VWP = 80              # padded per-head V block stride
F32 = mybir.dt.float32
F32R = mybir.dt.float32r
BF16 = mybir.dt.bfloat16
EXP = mybir.ActivationFunctionType.Exp

_CACHE = {}


def build_program(mm_dt=BF16):
    MMDT = mm_dt
    nc = bacc.Bacc("TRN2", target_bir_lowering=False, debug=False, num_devices=8)

    x_d = nc.dram_tensor("x", [N, C], MMDT, kind="ExternalInput")
    wqk_d = nc.dram_tensor("wqk", [C, 2 * NPAIR * P], MMDT, kind="ExternalInput")
    wv_d = nc.dram_tensor("wv", [C, HPC * HD], MMDT, kind="ExternalInput")
    wp_d = nc.dram_tensor("wp", [NPAIR * P, C], MMDT, kind="ExternalInput")
    bqk_d = nc.dram_tensor("bqk", [P, CT], F32, kind="ExternalInput")
    bv_d = nc.dram_tensor("bv", [1, HPC * HD], MMDT, kind="ExternalInput")
    out_d = nc.dram_tensor("out", [N, C], F32, kind="ExternalOutput")

    with tile.TileContext(nc) as tc, nc.allow_low_precision(
            reason="bf16 matmul pipeline, approx reciprocal for softmax denom"):
        with (
            tc.tile_pool(name="const", bufs=1) as cpool,
            tc.tile_pool(name="persist", bufs=1) as pp,
        ):
            # x^T first: everything downstream depends on it. Each block is
            # transposed in n-halves so the first v/k matmuls (which touch
            # only early n-tiles) can start after half the transfer.
            xT = [pp.tile([P, N], MMDT, name=f"xT{ct}", tag=f"xT{ct}")
                  for ct in range(CT)]
            for ct in range(CT):
                nc.sync.dma_start(xT[ct][:, 0:N // 2],
                                  x_d[0:N // 2, ct * P:(ct + 1) * P],
                                  transpose=True)
            wv_sb = []
            for ct in range(CT):
                tv = pp.tile([P, HPC * HD], MMDT, name=f"wv{ct}", tag=f"wv{ct}")
                nc.sync.dma_start(tv[:], wv_d[ct * P:(ct + 1) * P, :])
                wv_sb.append(tv)
            bv = cpool.tile([1, HPC * HD], MMDT, name="bv", tag="bv")
            nc.sync.dma_start(bv[:], bv_d[:])
            for ct in range(CT):
                nc.sync.dma_start(xT[ct][:, N // 2:N],
                                  x_d[N // 2:N, ct * P:(ct + 1) * P],
                                  transpose=True)
            w_sb = []
            for ct in range(CT):
                t = pp.tile([P, 2 * NPAIR * P], MMDT, name=f"w{ct}", tag=f"w{ct}")
                nc.sync.dma_start(t[:], wqk_d[ct * P:(ct + 1) * P, :])
                w_sb.append(t)
            bqk = cpool.tile([P, CT], F32, name="bqk", tag="bqk")
            nc.sync.dma_start(bqk[:], bqk_d[:])
            wp_sb = []
            for p in range(NPAIR):
                t = pp.tile([P, C], MMDT, name=f"wp{p}", tag=f"wp{p}")
                nc.sync.dma_start(t[:], wp_d[p * P:(p + 1) * P, :])
                wp_sb.append(t)

            onesb = cpool.tile([1, P], MMDT, name="onesb", tag="onesb")
            nc.gpsimd.memset(onesb[:], 1.0)

            qT = [pp.tile([P, N], MMDT, name=f"q{i}", tag=f"q{i}")
                  for i in range(NPAIR)]
            kT = [pp.tile([P, N], MMDT, name=f"k{i}", tag=f"k{i}")
                  for i in range(NPAIR)]
            v_sb = pp.tile([P, NT * HPC * VWP], MMDT, name="v", tag="v")
            # ones columns: memset everything to 1, V writes leave col 64 = 1
            nc.gpsimd.memset(v_sb[:], 1.0)
            o2_sb = [pp.tile([P, N], MMDT, name=f"o2{p}", tag=f"o2{p}")
                     for p in range(NPAIR)]

            # ---------------- phase A: q/k/v projections ----------------
            with (
                tc.tile_pool(name="vps", bufs=2, space="PSUM") as v_ps,
                tc.tile_pool(name="qkps", bufs=2, space="PSUM") as qk_ps,
            ):
                for j in range(4):
                    for ntl in range(4):
                        nt = 4 * j + ntl
                        # V rows for this n-tile (+ bias via K=1 matmul)
                        vps = v_ps.tile([P, HPC * HD], F32, name="vps", tag="vps")
                        for ct in range(CT):
                            nc.tensor.matmul(
                                vps[:],
                                xT[ct][:, nt * P:(nt + 1) * P],
                                wv_sb[ct][:],
                                start=(ct == 0), stop=False)
                        nc.tensor.matmul(
                            vps[:], onesb[0:1, :], bv[:], start=False, stop=True)
                        nc.vector.tensor_copy(
                            v_sb[:].rearrange("p (t w) -> p t w", w=VWP)
                                [:, nt * HPC:(nt + 1) * HPC, 0:HD],
                            vps[:].rearrange("p (h w) -> p h w", w=HD))
                    # q^T / k^T for this n-chunk of 512
                    for colt in range(2 * NPAIR):
                        qps = qk_ps.tile([P, 512], F32, name="qkp", tag="qkp")
                        for ct in range(CT):
                            nc.tensor.matmul(
                                qps[:],
                                w_sb[ct][:, colt * P:(colt + 1) * P],
                                xT[ct][:, j * 512:(j + 1) * 512],
                                start=(ct == 0), stop=(ct == CT - 1))
                        dest = qT[colt] if colt < NPAIR else kT[colt - NPAIR]
                        nc.vector.tensor_scalar_add(
                            dest[:, j * 512:(j + 1) * 512], qps[:],
                            bqk[:, colt:colt + 1])

            # ---------------- phase B: attention ----------------
            with (
                tc.tile_pool(name="sps", bufs=2, space="PSUM") as s_ps,
                tc.tile_pool(name="avps", bufs=2, space="PSUM") as av_ps,
                tc.tile_pool(name="exsb", bufs=3) as ex_pool,
                tc.tile_pool(name="o2u", bufs=4) as o2u_pool,
                tc.tile_pool(name="rec", bufs=4) as rec_pool,
            ):
                def attn_body(c, p):
                    """kt loop for q-chunk c, head pair p. Returns state for
                    the trailing normalization."""
                    av = [av_ps.tile([VW, QC], F32, name="av", tag="av")
                          for _ in range(2)]
                    for kt in range(NT):
                        sps = [s_ps.tile([P, QC], F32, name="s", tag="s")
                               for _ in range(2)]
                        # both heads' scores adjacent: row groups 0-63/64-127
                        # run concurrently on the PE
                        for half in range(2):
                            for par in range(2):
                                rows = slice(par * HD, (par + 1) * HD)
                                fsl = slice(half * 512, (half + 1) * 512)
                                nc.tensor.matmul(
                                    sps[par][:, fsl],
                                    kT[p][rows, kt * P:(kt + 1) * P],
                                    qT[p][rows, c * QC + half * 512:
                                          c * QC + (half + 1) * 512],
                                    start=True, stop=True)
                        exs = []
                        for par in range(2):
                            ex = ex_pool.tile([P, QC], MMDT, name="ex", tag="ex")
                            nc.scalar.activation(ex[:], sps[par][:], EXP)
                            exs.append(ex)
                        for par in range(2):
                            h = 2 * p + par
                            vcol = (kt * HPC + h) * VWP
                            for half in range(2):
                                fsl = slice(half * 512, (half + 1) * 512)
                                nc.tensor.matmul(
                                    av[par][:, fsl],
                                    v_sb[:, vcol:vcol + VW],
                                    exs[par][:, fsl],
                                    start=(kt == 0), stop=(kt == NT - 1))
                    # evacuate av fast on two engines; denominator row straight
                    # from PSUM so the normalize chain starts immediately
                    o2u, dens = [], []
                    for par in range(2):
                        eng = nc.scalar if par == 0 else nc.vector
                        den = rec_pool.tile([1, QC], F32, name="den", tag="den")
                        t = o2u_pool.tile([HD, QC], F32, name="o2u", tag="o2u")
                        if par == 0:
                            eng.copy(den[:], av[par][HD:VW, :])
                            eng.copy(t[:], av[par][0:HD, :])
                        else:
                            eng.tensor_copy(den[:], av[par][HD:VW, :])
                            eng.tensor_copy(t[:], av[par][0:HD, :])
                        dens.append(den)
                        o2u.append(t)
                    return o2u, dens

                def normalize(c, p, o2u, dens):
                    """Softmax-normalize o2u into o2_sb (pair-stacked bf16).
                    The reciprocal row is replicated across 64 partitions on
                    the (idle) gpsimd engine — no PE/PSUM involvement, so
                    nothing here can gate the attention pipeline."""
                    qsl = slice(c * QC, (c + 1) * QC)
                    for par in range(2):
                        rec = rec_pool.tile([1, QC], F32, name="rec", tag="rec")
                        nc.vector.reciprocal_approx_fast(rec[:], dens[par][:])
                        rbc = rec_pool.tile([HD, QC], F32, name="rbc",
                                            tag="rbc")
                        nc.gpsimd.partition_broadcast(rbc[:], rec[0:1, :])
                        nc.vector.tensor_tensor(
                            o2_sb[p][par * HD:(par + 1) * HD, qsl],
                            o2u[par][:], rbc[:],
                            op=mybir.AluOpType.mult)

                pending = None
                for c in range(NCH):
                    for p in range(NPAIR):
                        state = attn_body(c, p)
                        if pending is not None:
                            normalize(*pending)
                        pending = (c, p, *state)
                assert pending is not None
                normalize(*pending)

            # ---------------- phase C: out-projection ----------------
            with (
                tc.tile_pool(name="pps", bufs=2, space="PSUM") as p_ps,
                tc.tile_pool(name="outsb", bufs=2) as out_pool,
            ):
                for j in range(4):
                    out4 = out_pool.tile([P, 4 * C], F32, name="out4",
                                         tag="out4")
                    for tl in range(4):
                        t = 4 * j + tl
                        pps = p_ps.tile([P, C], F32, name="pps", tag="pps")
                        for n0, nw in ((0, 512), (512, C - 512)):
                            for p in range(NPAIR):
                                nc.tensor.matmul(
                                    pps[:, n0:n0 + nw],
                                    o2_sb[p][:, t * P:(t + 1) * P],
                                    wp_sb[p][:, n0:n0 + nw],
                                    start=(p == 0), stop=(p == NPAIR - 1))
                        nc.vector.tensor_copy(
                            out4[:, tl * C:(tl + 1) * C], pps[:])
                    nc.sync.dma_start(
                        out_d[j * 512:(j + 1) * 512, :]
                        .rearrange("(a p) c -> p a c", p=P),
                        out4[:].rearrange("p (a c) -> p a c", c=C))

    nc.compile()
    return nc


def _get_program(mm_dt=BF16):
    key = str(mm_dt)
    if key not in _CACHE:
        _CACHE[key] = build_program(mm_dt)
    return _CACHE[key]


def make_in_maps(x, w_qkv, b_qkv, w_proj):
    import ml_dtypes
    bf = ml_dtypes.bfloat16
    x = np.asarray(x, np.float32)
    w_qkv = np.asarray(w_qkv, np.float32)
    b_qkv = np.asarray(b_qkv, np.float32)
    w_proj = np.asarray(w_proj, np.float32)
    in_maps = []
    for c in range(8):
        b, hg = divmod(c, 2)
        hsl = slice(hg * HPC * HD, (hg + 1) * HPC * HD)
        wq = w_qkv[:, 0:C][:, hsl] * SCALE
        wk = w_qkv[:, C:2 * C][:, hsl]
        wv = w_qkv[:, 2 * C:3 * C][:, hsl]
        wqk_in = np.ascontiguousarray(
            np.concatenate([wq, wk], axis=1).astype(bf))
        bq = b_qkv[0:C][hsl] * SCALE
        bk = b_qkv[C:2 * C][hsl]
        bvv = b_qkv[2 * C:3 * C][hsl]
        bqk_in = np.ascontiguousarray(
            np.concatenate([bq, bk]).reshape(CT, P).T)
        wp_in = np.ascontiguousarray(w_proj[hsl, :].astype(bf))
        in_maps.append({
            "x": np.ascontiguousarray(x[b].astype(bf)),
            "wqk": wqk_in,
            "wv": np.ascontiguousarray(wv.astype(bf)),
            "wp": wp_in,
            "bqk": bqk_in,
            "bv": np.ascontiguousarray(bvv.reshape(1, HPC * HD).astype(bf)),
        })
    return in_maps


def run(x, w_qkv, b_qkv, w_proj, b_proj, mm_dt=BF16, **run_kwargs):
    nc = _get_program(mm_dt)
    in_maps = make_in_maps(x, w_qkv, b_qkv, w_proj)
    res = bass_utils.run_bass_kernel_spmd(
        nc, in_maps, core_ids=list(range(8)), **run_kwargs)
    y = np.empty((B, N, C), np.float32)
    for b in range(B):
        y[b] = res.results[2 * b]["out"] + res.results[2 * b + 1]["out"]
    y += np.asarray(b_proj, np.float32)
    return y, res


def kernel(x, w_qkv, b_qkv, w_proj, b_proj):
    y, _ = run(x, w_qkv, b_qkv, w_proj, b_proj)
    return y
